# revision 2
# baseline (speedup 1.0000x reference)
"""GAT residual block (nn_GATResBlock) on 8 Trainium2 NeuronCores.

Strategy (v2 — hardware-DGE indirect gathers)
---------------------------------------------
- Shard destination nodes (and their incoming edges) across the 8 cores;
  each core owns a contiguous range of 6250 dst nodes (49 blocks of 128).
- Host-side graph preprocessing: per (core, block), sort edges, build
  per-128-edge-tile gather index columns (int32 node-table rows) for both
  the edge's src node (xp | a_src) and dst node (a_dst).
- Algebraic folds: a_src/a_dst come out of the projection matmul via
  folded weights (Wcat = [W.T | W.T@att_src | W.T@att_dst]); the softmax
  max-trick is dropped (logits bounded, softmax shift-invariant) and the
  normalization happens at node level: agg = (sum ex*xp[src]) / (sum ex).
- Device: phase 1 builds a DRAM node table T1[row] = [xp | a_src | a_dst]
  (bf16, 136 cols); phase 3 uses `indirect_dma_start` (hardware descriptor
  generation, one [128,1] offset column per 128-edge tile) to gather the
  src rows and the dst a_dst slices — no software-DGE descriptor loop.
  A one-hot (edge,dst) matrix built with is_equal reduces weighted
  messages + softmax denominators in a PSUM-accumulated matmul chain.
- Pad slots index table row 0, whose a_src/a_dst are -1e30 => ex == 0.
"""

import sys
import types

sys.path.insert(0, "/opt/trn_rl_repo")

import numpy as np
import ml_dtypes

BFNP = ml_dtypes.bfloat16


# ---------------------------------------------------------------------------
# NTFF profile hook (missing antenv.axon_hooks in this image). Needed only
# when tracing; harmless otherwise.
def _install_ntff_hook():
    if "antenv.axon_hooks" in sys.modules:
        return
    try:
        hooks = types.ModuleType("antenv.axon_hooks")
        _h = [None]
        hooks.set_axon_ntff_profile_hook = lambda h: _h.__setitem__(0, h)
        hooks.get_axon_ntff_profile_hook = lambda: _h[0]
        sys.modules["antenv.axon_hooks"] = hooks
        import antenv

        antenv.axon_hooks = hooks
        from trn_agent_boot.trn_boot import _ntff_profile_via_ctypes

        hooks.set_axon_ntff_profile_hook(
            _ntff_profile_via_ctypes("/opt/axon/libaxon_pjrt.so")
        )
    except Exception:
        pass


_install_ntff_hook()

from concourse import bacc, bass, mybir, tile  # noqa: E402
from concourse.bass_utils import run_bass_kernel_spmd  # noqa: E402

F32 = mybir.dt.float32
BF16 = mybir.dt.bfloat16
I32 = mybir.dt.int32
ALU = mybir.AluOpType
ACTF = mybir.ActivationFunctionType

P = 128
NEG_SLOPE = 0.2
NEG_BIG = -1.0e30
ROWW = 136  # T1 cols: xp(128) + a_src(4) + a_dst(4)


class Cfg:
    def __init__(self, N=50000, IN=128, H=4, C=32, E=800000, NC=8):
        self.N, self.IN, self.H, self.C, self.E, self.NC = N, IN, H, C, E, NC
        self.HC = H * C
        assert self.HC == 128 and IN == 128
        assert N % NC == 0
        self.NLOC = N // NC                      # owned dst nodes per core
        self.NBLK = (self.NLOC + P - 1) // P     # dst blocks per core
        self.NLOCP = self.NBLK * P               # padded local nodes
        self.NR = ((1 + N + P - 1) // P) * P     # node-table rows (padded)
        self.TBS = None                          # per-block tile counts
        self.OFFS = None                         # per-block col offsets

    @property
    def TOTT(self):
        return int(sum(self.TBS))

    @property
    def TMAX(self):
        return int(max(self.TBS))


# ---------------------------------------------------------------------------
# Host-side preprocessing: edge partitioning + gather index construction.


def preprocess(cfg, edge_index):
    """Build per-core gather index arrays from the (2, E) edge list."""
    src = np.asarray(edge_index[0], dtype=np.int64)
    dst = np.asarray(edge_index[1], dtype=np.int64)
    core = dst // cfg.NLOC
    dstl = dst - core * cfg.NLOC
    blk = dstl // P
    dloc_v = dstl - blk * P

    order = np.lexsort((src, blk, core))
    core_s, blk_s = core[order], blk[order]
    src_s, dst_s, dloc_s = src[order], dst[order], dloc_v[order]

    gid = core_s * cfg.NBLK + blk_s
    ngroups = cfg.NC * cfg.NBLK
    counts = np.bincount(gid, minlength=ngroups).reshape(cfg.NC, cfg.NBLK)
    starts = np.concatenate(([0], np.cumsum(counts.reshape(-1))[:-1]))
    rank = np.arange(len(gid)) - starts[gid]

    # SPMD: one program for all cores -> per-block tile count is the max
    # over cores of ceil(count/128).
    tbs = np.maximum(1, -(-counts.max(axis=0) // P))       # [NBLK]
    offs = np.concatenate(([0], np.cumsum(tbs)[:-1]))      # [NBLK]
    cfg.TBS, cfg.OFFS = tbs.tolist(), offs.tolist()
    tott = cfg.TOTT

    srcidx = np.zeros((cfg.NC, P, tott), dtype=np.int32)
    dstidx = np.zeros((cfg.NC, P, tott), dtype=np.int32)
    dloc = np.zeros((cfg.NC, P, tott), dtype=np.float32)

    t = rank // P
    p = rank % P
    col = offs[blk_s] + t
    srcidx[core_s, p, col] = src_s + 1
    dstidx[core_s, p, col] = dst_s + 1
    dloc[core_s, p, col] = dloc_s

    return [dict(srcidx=np.ascontiguousarray(srcidx[c]),
                 dstidx=np.ascontiguousarray(dstidx[c]),
                 dloc=np.ascontiguousarray(dloc[c]))
            for c in range(cfg.NC)]


def make_weights(cfg, W, att_src, att_dst, bias, skip_W, skip_b):
    """Fold attention vectors into the projection weights."""
    H, C, IN = cfg.H, cfg.C, cfg.IN
    A_s = np.zeros((IN, H), dtype=np.float32)
    A_d = np.zeros((IN, H), dtype=np.float32)
    for h in range(H):
        # a_src[n,h] = sum_c xp[n,h*C+c]*att_src[h,c] = x @ (W[h*C:+C].T @ att)
        A_s[:, h] = W[h * C:(h + 1) * C, :].T @ att_src[0, h]
        A_d[:, h] = W[h * C:(h + 1) * C, :].T @ att_dst[0, h]
    Wcat = np.concatenate([W.T, A_s, A_d], axis=1).astype(BFNP)   # [IN,136]
    Wsk = np.ascontiguousarray(skip_W.T.astype(BFNP))             # [IN,128]
    bias2 = np.tile((bias + skip_b).astype(np.float32)[None, :], (P, 1))
    return Wcat, Wsk, bias2


def make_inputs(cfg, x, edge_index, W, att_src, att_dst, bias, skip_W, skip_b):
    per_core_idx = preprocess(cfg, edge_index)
    Wcat, Wsk, bias2 = make_weights(cfg, W, att_src, att_dst, bias, skip_W,
                                    skip_b)
    xT = np.zeros((cfg.IN, cfg.NR), dtype=BFNP)
    xT[:, 1:1 + cfg.N] = np.asarray(x, dtype=np.float32).T.astype(BFNP)
    iota = np.tile(np.arange(P, dtype=np.float32)[None, :], (P, 1))
    negr = np.full((1, 8), NEG_BIG, dtype=np.float32)

    in_maps = []
    for c in range(cfg.NC):
        xTl = np.zeros((cfg.IN, cfg.NLOCP), dtype=BFNP)
        xTl[:, :cfg.NLOC] = np.asarray(
            x[c * cfg.NLOC:(c + 1) * cfg.NLOC],
            dtype=np.float32).T.astype(BFNP)
        m = dict(xT=xT, xTl=np.ascontiguousarray(xTl), Wcat=Wcat, Wsk=Wsk,
                 bias2=bias2, iota=iota, negr=negr, **per_core_idx[c])
        in_maps.append(m)
    return in_maps


# ---------------------------------------------------------------------------
# Device program.


def build_program(cfg):
    """Build the per-core SPMD Bass program."""
    nc = bacc.Bacc(None)
    NBLK, NR = cfg.NBLK, cfg.NR
    TBS, OFFS, TOTT, TMAX = cfg.TBS, cfg.OFFS, cfg.TOTT, cfg.TMAX

    xT = nc.declare_dram_parameter("xT", [cfg.IN, NR], BF16, isOutput=False)
    xTl = nc.declare_dram_parameter("xTl", [cfg.IN, cfg.NLOCP], BF16,
                                    isOutput=False)
    Wcat = nc.declare_dram_parameter("Wcat", [cfg.IN, ROWW], BF16,
                                     isOutput=False)
    Wsk = nc.declare_dram_parameter("Wsk", [cfg.IN, 128], BF16,
                                    isOutput=False)
    bias2 = nc.declare_dram_parameter("bias2", [P, 128], F32, isOutput=False)
    iota = nc.declare_dram_parameter("iota", [P, P], F32, isOutput=False)
    negr = nc.declare_dram_parameter("negr", [1, 8], F32, isOutput=False)
    srcidx = nc.declare_dram_parameter("srcidx", [P, TOTT], I32,
                                       isOutput=False)
    dstidx = nc.declare_dram_parameter("dstidx", [P, TOTT], I32,
                                       isOutput=False)
    dloc = nc.declare_dram_parameter("dloc", [P, TOTT], F32, isOutput=False)
    out = nc.declare_dram_parameter("out", [cfg.NLOCP, 128], F32,
                                    isOutput=True)

    T1 = nc.dram_tensor("T1", [NR, ROWW], BF16)

    with tile.TileContext(nc) as tc:
        with (
            tc.tile_pool(name="const", bufs=1) as cpool,
            tc.tile_pool(name="prol", bufs=4) as prol,
            tc.tile_pool(name="gp", bufs=4) as gp,
            tc.tile_pool(name="main", bufs=4) as mp,
            tc.tile_pool(name="epi", bufs=2) as ep,
        ):
            # ---- constants ----
            iota_sb = cpool.tile([P, P], F32)
            nc.sync.dma_start(out=iota_sb[:], in_=iota[:])
            wcat_sb = cpool.tile([P, ROWW], BF16)
            nc.sync.dma_start(out=wcat_sb[:], in_=Wcat[:])
            wsk_sb = cpool.tile([P, 128], BF16)
            nc.sync.dma_start(out=wsk_sb[:], in_=Wsk[:])
            bias_sb = cpool.tile([P, 128], F32)
            nc.sync.dma_start(out=bias_sb[:], in_=bias2[:])
            negr_sb = cpool.tile([1, 8], F32)
            nc.sync.dma_start(out=negr_sb[:], in_=negr[:])
            srcidx_sb = cpool.tile([P, TOTT], I32)
            nc.sync.dma_start(out=srcidx_sb[:], in_=srcidx[:])
            dstidx_sb = cpool.tile([P, TOTT], I32)
            nc.sync.dma_start(out=dstidx_sb[:], in_=dstidx[:])
            dloc_sb = cpool.tile([P, TOTT], F32)
            nc.sync.dma_start(out=dloc_sb[:], in_=dloc[:])
            skip_sb = cpool.tile([P, NBLK * 128], F32)

            # ---- phase 1: global node table T1 = [xp | a_src | a_dst] ----
            with tc.tile_pool(name="pp", bufs=2, space="PSUM") as pp:
                CH = 4
                for i0 in range(0, NR // P, CH):
                    ch = min(CH, NR // P - i0)
                    xt = prol.tile([P, CH * P], BF16, tag="xt")
                    nc.sync.dma_start(
                        out=xt[:, 0:ch * P], in_=xT[:, i0 * P:(i0 + ch) * P])
                    st4 = prol.tile([P, CH, ROWW], BF16, tag="st4")
                    for k in range(ch):
                        ps = pp.tile([P, ROWW], F32)
                        nc.tensor.matmul(out=ps[:],
                                         lhsT=xt[:, k * P:(k + 1) * P],
                                         rhs=wcat_sb[:], start=True, stop=True)
                        nc.scalar.activation(out=st4[:, k, :], in_=ps[:],
                                             func=ACTF.Copy)
                    nc.scalar.dma_start(
                        out=T1[i0 * P:(i0 + ch) * P, :].rearrange(
                            "(k p) c -> p k c", p=P),
                        in_=st4[:, 0:ch, :])
                # pad row 0: a_src/a_dst = NEG_BIG so padded edges get ex = 0
                negr_bf = cpool.tile([1, 8], BF16)
                nc.vector.tensor_copy(out=negr_bf[:], in_=negr_sb[:])
                nc.sync.dma_start(out=T1[0:1, 128:136], in_=negr_bf[:])

                # ---- phase 2: local skip projection (SBUF) ----
                for j in range(NBLK):
                    xlb = prol.tile([P, P], BF16, tag="xlb")
                    nc.sync.dma_start(out=xlb[:],
                                      in_=xTl[:, j * P:(j + 1) * P])
                    ps2 = pp.tile([P, 128], F32, tag="ps2")
                    nc.tensor.matmul(out=ps2[:], lhsT=xlb[:], rhs=wsk_sb[:],
                                     start=True, stop=True)
                    nc.vector.tensor_tensor(out=skip_sb[:, j * P:(j + 1) * P],
                                            in0=ps2[:], in1=bias_sb[:],
                                            op=ALU.add)

            # ---- phase 3: per-dst-block edge processing ----
            with tc.tile_pool(name="acc", bufs=2, space="PSUM") as ap:
                for b in range(NBLK):
                    TB, off = TBS[b], OFFS[b]
                    G1 = gp.tile([P, TMAX, ROWW], BF16, tag="G1")
                    G2 = gp.tile([P, TMAX, 4], BF16, tag="G2")
                    for t in range(TB):
                        nc.gpsimd.indirect_dma_start(
                            out=G1[:, t, :],
                            out_offset=None,
                            in_=T1[:],
                            in_offset=bass.IndirectOffsetOnAxis(
                                ap=srcidx_sb[:, off + t:off + t + 1], axis=0),
                        )
                    for t in range(TB):
                        nc.gpsimd.indirect_dma_start(
                            out=G2[:, t, :],
                            out_offset=None,
                            in_=T1[:],
                            in_offset=bass.IndirectOffsetOnAxis(
                                ap=dstidx_sb[:, off + t:off + t + 1], axis=0),
                            element_offset=132,
                        )

                    # one-hot S[e, d] = (dloc[e] == d), bf16
                    S = mp.tile([P, TMAX, P], BF16, tag="S")
                    nc.vector.tensor_tensor(
                        out=S[:, 0:TB, :],
                        in0=dloc_sb[:, off:off + TB,
                                    None].to_broadcast([P, TB, P]),
                        in1=iota_sb[:, None, :].to_broadcast([P, TB, P]),
                        op=ALU.is_equal,
                    )
                    # logits + leaky relu + exp
                    elog = mp.tile([P, TMAX, 4], F32, tag="elog")
                    nc.vector.tensor_tensor(
                        out=elog[:, 0:TB, :],
                        in0=G1[:, 0:TB, 128:132],
                        in1=G2[:, 0:TB, :],
                        op=ALU.add)
                    el2 = mp.tile([P, TMAX, 4], F32, tag="el2")
                    nc.vector.scalar_tensor_tensor(
                        out=el2[:, 0:TB, :], in0=elog[:, 0:TB, :],
                        scalar=NEG_SLOPE, in1=elog[:, 0:TB, :],
                        op0=ALU.mult, op1=ALU.max)
                    ex = mp.tile([P, TMAX, 4], F32, tag="ex")
                    nc.scalar.activation(out=ex[:, 0:TB, :],
                                         in_=el2[:, 0:TB, :], func=ACTF.Exp)
                    # V = [ex * xp | ex]  (bf16)
                    V = mp.tile([P, TMAX, 132], BF16, tag="V")
                    nc.vector.tensor_tensor(
                        out=V[:, 0:TB, 0:128].rearrange(
                            "p t (h c) -> p t h c", c=32),
                        in0=G1[:, 0:TB, 0:128].rearrange(
                            "p t (h c) -> p t h c", c=32),
                        in1=ex[:, 0:TB, :, None].to_broadcast([P, TB, 4, 32]),
                        op=ALU.mult,
                    )
                    nc.scalar.activation(out=V[:, 0:TB, 128:132],
                                         in_=ex[:, 0:TB, :], func=ACTF.Copy)
                    acc = ap.tile([P, 132], F32)
                    for t in range(TB):
                        nc.tensor.matmul(out=acc[:], lhsT=S[:, t, :],
                                         rhs=V[:, t, :], start=(t == 0),
                                         stop=(t == TB - 1))
                    # epilogue: divide, + skip, ELU
                    dn = ep.tile([P, 4], F32, tag="dn")
                    nc.vector.tensor_scalar_add(out=dn[:],
                                                in0=acc[:, 128:132],
                                                scalar1=1e-6)
                    rcp = ep.tile([P, 4], F32, tag="rcp")
                    nc.vector.reciprocal(out=rcp[:], in_=dn[:])
                    y = ep.tile([P, 128], F32, tag="y")
                    nc.vector.tensor_tensor(
                        out=y[:].rearrange("p (h c) -> p h c", c=32),
                        in0=acc[:, 0:128].rearrange("p (h c) -> p h c", c=32),
                        in1=rcp[:, :, None].to_broadcast([P, 4, 32]),
                        op=ALU.mult,
                    )
                    y2 = ep.tile([P, 128], F32, tag="y2")
                    nc.vector.tensor_tensor(
                        out=y2[:], in0=y[:],
                        in1=skip_sb[:, b * P:(b + 1) * P], op=ALU.add)
                    # elu(v) = max(v,0) + exp(min(v,0)) - 1
                    mn = ep.tile([P, 128], F32, tag="mn")
                    nc.vector.tensor_scalar_min(out=mn[:], in0=y2[:],
                                                scalar1=0.0)
                    e1 = ep.tile([P, 128], F32, tag="e1")
                    nc.scalar.activation(out=e1[:], in_=mn[:], func=ACTF.Exp)
                    mx = ep.tile([P, 128], F32, tag="mx")
                    nc.vector.tensor_scalar_max(out=mx[:], in0=y2[:],
                                                scalar1=0.0)
                    yo = ep.tile([P, 128], F32, tag="yo")
                    nc.vector.scalar_tensor_tensor(
                        out=yo[:], in0=mx[:], scalar=-1.0, in1=e1[:],
                        op0=ALU.add, op1=ALU.add)
                    nc.scalar.dma_start(out=out[b * P:(b + 1) * P, :],
                                        in_=yo[:])

    nc.compile()
    return nc


# ---------------------------------------------------------------------------
# Public entry point.

_CACHE = {}


def _get_program(cfg):
    key = (cfg.N, cfg.E, cfg.NC, tuple(cfg.TBS))
    if key not in _CACHE:
        _CACHE[key] = build_program(cfg)
    return _CACHE[key]


def run_full(inputs, trace=False, **spmd_kwargs):
    cfg = Cfg()
    in_maps = make_inputs(cfg, **{k: np.asarray(v) for k, v in inputs.items()})
    nc = _get_program(cfg)
    res = run_bass_kernel_spmd(nc, in_maps, list(range(cfg.NC)), trace=trace,
                               **spmd_kwargs)
    outs = [res.results[c]["out"][:cfg.NLOC] for c in range(cfg.NC)]
    return np.concatenate(outs, axis=0).astype(np.float32), res


def kernel(x, edge_index, W, att_src, att_dst, bias, skip_W, skip_b):
    out, _ = run_full(dict(x=x, edge_index=edge_index, W=W, att_src=att_src,
                           att_dst=att_dst, bias=bias, skip_W=skip_W,
                           skip_b=skip_b))
    return out


# revision 7
# speedup vs baseline: 2.2814x; 2.2814x over previous
"""GAT residual block (nn_GATResBlock) on 8 Trainium2 NeuronCores.

Strategy (v3 — SWDGE gather, trimmed + offloaded)
-------------------------------------------------
- Shard destination nodes (and their incoming edges) across the 8 cores;
  each core owns a contiguous range of 6250 dst nodes (49 blocks of 128).
- Host-side graph preprocessing: per (core, block), edges sorted and split
  into A/B halves by src row (int16 gather index range is 32768 rows),
  padded to 128-edge tiles (per-block ragged tile counts, max over cores,
  so descriptor-generation time tracks the real edge count); pad slots
  gather node row 0 and are masked out of the one-hot aggregation matrix
  via dloc = -1.
- Algebraic folds: a_src = x @ (W.T @ att_src-expanded) comes out of the
  projection matmul; the segment-softmax max-trick is dropped (logits are
  bounded, softmax is shift invariant) and normalization happens at node
  level: agg = (sum ex*xp[src]) / (sum ex).
- Device per core: one replicated projection pass (bf16 inputs, scalar-
  engine PSUM evacuation) builds a DRAM node table T1[row] = [xp | a_src];
  per dst-block, dma_gather fetches the rows of the block's source nodes,
  a one-hot (edge,dst) selection matrix built with iota/is_equal feeds a
  PSUM-accumulated matmul that reduces weighted messages + softmax
  denominators in one pass; a_dst reaches the edges via per-tile
  transposes of the one-hot matrix (PSUM copies on the scalar engine) and
  tiny matmuls. Epilogue divides, adds the skip projection, applies ELU.
"""

import sys
import types

sys.path.insert(0, "/opt/trn_rl_repo")

import numpy as np
import ml_dtypes

BFNP = ml_dtypes.bfloat16


# ---------------------------------------------------------------------------
# NTFF profile hook (missing antenv.axon_hooks in this image). Needed only
# when tracing; harmless otherwise.
def _install_ntff_hook():
    if "antenv.axon_hooks" in sys.modules:
        return
    try:
        hooks = types.ModuleType("antenv.axon_hooks")
        _h = [None]
        hooks.set_axon_ntff_profile_hook = lambda h: _h.__setitem__(0, h)
        hooks.get_axon_ntff_profile_hook = lambda: _h[0]
        sys.modules["antenv.axon_hooks"] = hooks
        import antenv

        antenv.axon_hooks = hooks
        from trn_agent_boot.trn_boot import _ntff_profile_via_ctypes

        hooks.set_axon_ntff_profile_hook(
            _ntff_profile_via_ctypes("/opt/axon/libaxon_pjrt.so")
        )
    except Exception:
        pass


_install_ntff_hook()

from concourse import bacc, mybir, tile  # noqa: E402
from concourse.bass_utils import run_bass_kernel_spmd  # noqa: E402

F32 = mybir.dt.float32
BF16 = mybir.dt.bfloat16
I16 = mybir.dt.int16
ALU = mybir.AluOpType
ACTF = mybir.ActivationFunctionType

P = 128
NEG_SLOPE = 0.2
ROWW = 256  # T1 bf16 cols: xp(128) + a_src(4) + pad (elem must be 256B mult)


class Cfg:
    def __init__(self, N=50000, IN=128, H=4, C=32, E=800000, NC=8,
                 SPLIT=32768):
        self.N, self.IN, self.H, self.C, self.E, self.NC = N, IN, H, C, E, NC
        self.HC = H * C
        assert self.HC == 128 and IN == 128
        assert N % NC == 0
        self.NLOC = N // NC                      # owned dst nodes per core
        self.NBLK = (self.NLOC + P - 1) // P     # dst blocks per core
        self.NLOCP = self.NBLK * P               # padded local nodes
        self.SPLIT = SPLIT                       # int16 A/B table split
        self.NR = ((N + P - 1) // P) * P         # node-table rows (padded)
        assert self.NR - SPLIT <= 32768
        self.TAS = None   # per-block A tiles
        self.TBS = None   # per-block B tiles


# ---------------------------------------------------------------------------
# Host-side preprocessing: edge partitioning + gather index construction.


def _wrap_idx(arr):
    """[K*128] edge-slot array -> [128, K*8] int16 'wrapped' index layout
    (index i lives at [i % 16, i // 16], replicated across the 8 groups)."""
    k16 = arr.reshape(-1, 16).T.astype(np.int16)  # [16, K*8]
    return np.tile(k16, (8, 1))                   # [128, K*8]


def preprocess(cfg, edge_index):
    """Build per-core gather index arrays from the (2, E) edge list."""
    src = np.asarray(edge_index[0], dtype=np.int64)
    dst = np.asarray(edge_index[1], dtype=np.int64)
    core = dst // cfg.NLOC
    dstl = dst - core * cfg.NLOC
    blk = dstl // P
    isB = (src >= cfg.SPLIT).astype(np.int64)

    order = np.lexsort((src, isB, blk, core))
    core_s, blk_s, isB_s = core[order], blk[order], isB[order]
    src_s, dstl_s = src[order], dstl[order]

    gid = ((core_s * cfg.NBLK) + blk_s) * 2 + isB_s
    ngroups = cfg.NC * cfg.NBLK * 2
    counts = np.bincount(gid, minlength=ngroups)
    starts = np.concatenate(([0], np.cumsum(counts)[:-1]))
    rank = np.arange(len(gid)) - starts[gid]

    cAB = counts.reshape(cfg.NC, cfg.NBLK, 2)
    tas = np.maximum(1, -(-cAB[:, :, 0].max(axis=0) // P))   # [NBLK]
    tbs = np.maximum(1, -(-cAB[:, :, 1].max(axis=0) // P))
    cfg.TAS, cfg.TBS = tas.tolist(), tbs.tolist()
    aoff = np.concatenate(([0], np.cumsum(tas)[:-1]))
    boff = np.concatenate(([0], np.cumsum(tbs)[:-1]))
    doff = np.concatenate(([0], np.cumsum(tas + tbs)[:-1]))
    SA, SB = int(tas.sum()), int(tbs.sum())
    ST = SA + SB

    idxA = np.zeros((cfg.NC, SA * P), dtype=np.int64)
    idxB = np.zeros((cfg.NC, SB * P), dtype=np.int64)
    dloc = np.full((cfg.NC, P, ST), -1.0, dtype=np.float32)

    a = isB_s == 0
    idxA[core_s[a], aoff[blk_s[a]] * P + rank[a]] = src_s[a]
    ta = rank[a] // P
    dloc[core_s[a], rank[a] % P, doff[blk_s[a]] + ta] = \
        dstl_s[a] - blk_s[a] * P
    b = ~a
    idxB[core_s[b], boff[blk_s[b]] * P + rank[b]] = src_s[b] - cfg.SPLIT
    tb = rank[b] // P
    dloc[core_s[b], rank[b] % P,
         doff[blk_s[b]] + tas[blk_s[b]] + tb] = dstl_s[b] - blk_s[b] * P

    per_core = []
    for c in range(cfg.NC):
        wA = np.concatenate(
            [_wrap_idx(idxA[c, aoff[b2] * P:(aoff[b2] + tas[b2]) * P])
             for b2 in range(cfg.NBLK)], axis=1)
        wB = np.concatenate(
            [_wrap_idx(idxB[c, boff[b2] * P:(boff[b2] + tbs[b2]) * P])
             for b2 in range(cfg.NBLK)], axis=1)
        per_core.append(dict(idxA=np.ascontiguousarray(wA),
                             idxB=np.ascontiguousarray(wB),
                             dloc=np.ascontiguousarray(dloc[c])))
    return per_core


def make_weights(cfg, W, att_src, att_dst, bias, skip_W, skip_b):
    """Fold attention vectors into the projection weights."""
    H, C, IN = cfg.H, cfg.C, cfg.IN
    A_s = np.zeros((IN, H), dtype=np.float32)
    A_d = np.zeros((IN, H), dtype=np.float32)
    for h in range(H):
        # a_src[n,h] = sum_c xp[n,h*C+c]*att_src[h,c] = x @ (W[h*C:+C].T @ att)
        A_s[:, h] = W[h * C:(h + 1) * C, :].T @ att_src[0, h]
        A_d[:, h] = W[h * C:(h + 1) * C, :].T @ att_dst[0, h]
    Wcat = np.concatenate([W.T, A_s], axis=1).astype(BFNP)       # [IN,132]
    Wsk = np.concatenate([skip_W.T, A_d], axis=1).astype(BFNP)   # [IN,132]
    bias2 = np.tile((bias + skip_b).astype(np.float32)[None, :], (P, 1))
    return Wcat, Wsk, bias2


def make_inputs(cfg, x, edge_index, W, att_src, att_dst, bias, skip_W, skip_b):
    per_core_idx = preprocess(cfg, edge_index)
    Wcat, Wsk, bias2 = make_weights(cfg, W, att_src, att_dst, bias, skip_W,
                                    skip_b)
    xT = np.zeros((cfg.IN, cfg.NR), dtype=BFNP)
    xT[:, 0:cfg.N] = np.asarray(x, dtype=np.float32).T.astype(BFNP)
    iota = np.tile(np.arange(P, dtype=np.float32)[None, :], (P, 1))
    iotap = np.tile(np.arange(P, dtype=np.float32)[:, None], (1, P))

    in_maps = []
    for c in range(cfg.NC):
        xTl = np.zeros((cfg.IN, cfg.NLOCP), dtype=BFNP)
        xTl[:, :cfg.NLOC] = np.asarray(
            x[c * cfg.NLOC:(c + 1) * cfg.NLOC],
            dtype=np.float32).T.astype(BFNP)
        m = dict(xT=xT, xTl=np.ascontiguousarray(xTl), Wcat=Wcat, Wsk=Wsk,
                 bias2=bias2, iota=iota, iotap=iotap, **per_core_idx[c])
        in_maps.append(m)
    return in_maps


# ---------------------------------------------------------------------------
# Device program.


def build_program(cfg):
    """Build the per-core SPMD Bass program."""
    nc = bacc.Bacc(None)
    NBLK, NR = cfg.NBLK, cfg.NR
    TAS, TBS = cfg.TAS, cfg.TBS
    SA, SB = sum(TAS), sum(TBS)
    ST = SA + SB
    TMAX = max(TAS[b] + TBS[b] for b in range(NBLK))

    xT = nc.declare_dram_parameter("xT", [cfg.IN, NR], BF16, isOutput=False)
    xTl = nc.declare_dram_parameter("xTl", [cfg.IN, cfg.NLOCP], BF16,
                                    isOutput=False)
    Wcat = nc.declare_dram_parameter("Wcat", [cfg.IN, 132], BF16,
                                     isOutput=False)
    Wsk = nc.declare_dram_parameter("Wsk", [cfg.IN, 132], BF16,
                                    isOutput=False)
    bias2 = nc.declare_dram_parameter("bias2", [P, 128], F32, isOutput=False)
    iota = nc.declare_dram_parameter("iota", [P, P], F32, isOutput=False)
    iotap = nc.declare_dram_parameter("iotap", [P, P], F32, isOutput=False)
    idxA = nc.declare_dram_parameter("idxA", [P, SA * 8], I16, isOutput=False)
    idxB = nc.declare_dram_parameter("idxB", [P, SB * 8], I16, isOutput=False)
    dloc = nc.declare_dram_parameter("dloc", [P, ST], F32, isOutput=False)
    out = nc.declare_dram_parameter("out", [cfg.NLOCP, 128], F32,
                                    isOutput=True)

    T1 = nc.dram_tensor("T1", [NR, ROWW], BF16)

    with tile.TileContext(nc) as tc:
        with (
            tc.tile_pool(name="const", bufs=1) as cpool,
            tc.tile_pool(name="prol", bufs=4) as prol,
            tc.tile_pool(name="gbuf", bufs=4) as gb,
            tc.tile_pool(name="main", bufs=4) as mp,
            tc.tile_pool(name="epi", bufs=2) as ep,
        ):
            # ---- constants ----
            iota_sb = cpool.tile([P, P], F32)
            nc.sync.dma_start(out=iota_sb[:], in_=iota[:])
            iotap_sb = cpool.tile([P, P], F32)
            nc.sync.dma_start(out=iotap_sb[:], in_=iotap[:])
            ident_bf = cpool.tile([P, P], BF16)
            nc.vector.tensor_tensor(out=ident_bf[:], in0=iota_sb[:],
                                    in1=iotap_sb[:], op=ALU.is_equal)
            wcat_sb = cpool.tile([P, 132], BF16)
            nc.sync.dma_start(out=wcat_sb[:], in_=Wcat[:])
            wsk_sb = cpool.tile([P, 132], BF16)
            nc.sync.dma_start(out=wsk_sb[:], in_=Wsk[:])
            bias_sb = cpool.tile([P, 128], F32)
            nc.sync.dma_start(out=bias_sb[:], in_=bias2[:])
            idxA_sb = cpool.tile([P, SA * 8], I16)
            nc.sync.dma_start(out=idxA_sb[:], in_=idxA[:])
            idxB_sb = cpool.tile([P, SB * 8], I16)
            nc.sync.dma_start(out=idxB_sb[:], in_=idxB[:])
            dloc_sb = cpool.tile([P, ST], F32)
            nc.sync.dma_start(out=dloc_sb[:], in_=dloc[:])
            skip_sb = cpool.tile([P, NBLK * 128], F32)
            adst_sb = cpool.tile([P, NBLK * 4], BF16)

            # ---- phase 1: global node table T1 = [xp(bf16) | a_src] ----
            with tc.tile_pool(name="pp", bufs=2, space="PSUM") as pp:
                CH = 4
                for i0 in range(0, NR // P, CH):
                    ch = min(CH, NR // P - i0)
                    xt = prol.tile([P, CH * P], BF16, tag="xt")
                    nc.sync.dma_start(
                        out=xt[:, 0:ch * P], in_=xT[:, i0 * P:(i0 + ch) * P])
                    st4 = prol.tile([P, CH, 132], BF16, tag="st4")
                    for k in range(ch):
                        ps = pp.tile([P, 132], F32)
                        nc.tensor.matmul(out=ps[:],
                                         lhsT=xt[:, k * P:(k + 1) * P],
                                         rhs=wcat_sb[:], start=True, stop=True)
                        nc.vector.tensor_copy(out=st4[:, k, :], in_=ps[:])
                    nc.scalar.dma_start(
                        out=T1[i0 * P:(i0 + ch) * P, 0:132].rearrange(
                            "(k p) c -> p k c", p=P),
                        in_=st4[:, 0:ch, :])

                # ---- phase 2: local skip projection + a_dst (SBUF) ----
                for j in range(NBLK):
                    xlb = prol.tile([P, P], BF16, tag="xlb")
                    nc.sync.dma_start(out=xlb[:],
                                      in_=xTl[:, j * P:(j + 1) * P])
                    ps2 = pp.tile([P, 132], F32, tag="ps2")
                    nc.tensor.matmul(out=ps2[:], lhsT=xlb[:], rhs=wsk_sb[:],
                                     start=True, stop=True)
                    nc.vector.tensor_tensor(out=skip_sb[:, j * P:(j + 1) * P],
                                            in0=ps2[:, 0:128], in1=bias_sb[:],
                                            op=ALU.add)
                    nc.vector.tensor_copy(out=adst_sb[:, j * 4:(j + 1) * 4],
                                          in_=ps2[:, 128:132])

            # ---- phase 3: per-dst-block edge processing ----
            aoff = boff = doff = 0
            with (
                tc.tile_pool(name="acc", bufs=2, space="PSUM") as ap,
                tc.tile_pool(name="stp", bufs=2, space="PSUM") as sp,
                tc.tile_pool(name="adp", bufs=2, space="PSUM") as adp,
            ):
                for b in range(NBLK):
                    TA, TB = TAS[b], TBS[b]
                    T = TA + TB
                    G1 = gb.tile([P, TMAX, ROWW], BF16, tag="G1")
                    nc.gpsimd.dma_gather(
                        out_ap=G1[:, 0:TA, :],
                        in_ap=T1[:],
                        idxs_ap=idxA_sb[:, aoff * 8:(aoff + TA) * 8],
                        num_idxs=TA * P,
                        num_idxs_reg=TA * P,
                        elem_size=ROWW,
                        single_packet=False,
                    )
                    nc.gpsimd.dma_gather(
                        out_ap=G1[:, TA:T, :],
                        in_ap=T1[cfg.SPLIT:, :],
                        idxs_ap=idxB_sb[:, boff * 8:(boff + TB) * 8],
                        num_idxs=TB * P,
                        num_idxs_reg=TB * P,
                        elem_size=ROWW,
                        single_packet=False,
                    )

                    # one-hot S[e, d] = (dloc[e] == d), bf16; pads (-1) -> 0
                    S = mp.tile([P, TMAX, P], BF16, tag="S")
                    nc.vector.tensor_tensor(
                        out=S[:, 0:T, :],
                        in0=dloc_sb[:, doff:doff + T,
                                    None].to_broadcast([P, T, P]),
                        in1=iota_sb[:, None, :].to_broadcast([P, T, P]),
                        op=ALU.is_equal,
                    )
                    # a_dst -> per-edge via St = S^T + tiny matmul
                    elog = mp.tile([P, TMAX, 4], F32, tag="elog")
                    for t0 in range(0, T, 4):
                        tn = min(4, T - t0)
                        stps = sp.tile([P, 512], BF16, tag="stps")
                        for k in range(tn):
                            nc.tensor.transpose(
                                out=stps[:, k * P:(k + 1) * P],
                                in_=S[:, t0 + k, :], identity=ident_bf[:])
                        st4b = mp.tile([P, 4, P], BF16, tag="st4b")
                        nc.vector.tensor_copy(out=st4b[:, 0:tn, :],
                                              in_=stps[:, 0:tn * P].rearrange(
                                                  "p (t c) -> p t c", c=P))
                        adps = adp.tile([P, 16], F32, tag="adps")
                        for k in range(tn):
                            nc.tensor.matmul(
                                out=adps[:, k * 4:(k + 1) * 4],
                                lhsT=st4b[:, k, :],
                                rhs=adst_sb[:, b * 4:(b + 1) * 4],
                                start=True, stop=True)
                        for (u0, un, p0) in (
                                [(t0, tn, 0)] if (t0 >= TA or t0 + tn <= TA)
                                else [(t0, TA - t0, 0),
                                      (TA, t0 + tn - TA, TA - t0)]):
                            nc.vector.tensor_tensor(
                                out=elog[:, u0:u0 + un, :],
                                in0=G1[:, u0:u0 + un, 128:132],
                                in1=adps[:, p0 * 4:(p0 + un) * 4].rearrange(
                                    "p (t f) -> p t f", f=4),
                                op=ALU.add)
                    el2 = mp.tile([P, TMAX, 4], F32, tag="el2")
                    nc.vector.scalar_tensor_tensor(
                        out=el2[:, 0:T, :], in0=elog[:, 0:T, :],
                        scalar=NEG_SLOPE, in1=elog[:, 0:T, :],
                        op0=ALU.mult, op1=ALU.max)
                    ex = mp.tile([P, TMAX, 4], F32, tag="ex")
                    nc.scalar.activation(out=ex[:, 0:T, :], in_=el2[:, 0:T, :],
                                         func=ACTF.Exp)
                    # V = [ex * xp | ex]  (bf16)
                    V = mp.tile([P, TMAX, 132], BF16, tag="V")
                    nc.vector.tensor_tensor(
                        out=V[:, 0:T, 0:128].rearrange(
                            "p t (h c) -> p t h c", c=32),
                        in0=G1[:, 0:T, 0:128].rearrange(
                            "p t (h c) -> p t h c", c=32),
                        in1=ex[:, 0:T, :, None].to_broadcast([P, T, 4, 32]),
                        op=ALU.mult,
                    )
                    nc.vector.tensor_copy(out=V[:, 0:T, 128:132],
                                          in_=ex[:, 0:T, :])
                    acc = ap.tile([P, 132], F32)
                    for t in range(T):
                        nc.tensor.matmul(out=acc[:], lhsT=S[:, t, :],
                                         rhs=V[:, t, :], start=(t == 0),
                                         stop=(t == T - 1))
                    # epilogue: divide, + skip, ELU
                    dn = ep.tile([P, 4], F32, tag="dn")
                    nc.vector.tensor_scalar_add(out=dn[:],
                                                in0=acc[:, 128:132],
                                                scalar1=1e-6)
                    rcp = ep.tile([P, 4], F32, tag="rcp")
                    nc.vector.reciprocal(out=rcp[:], in_=dn[:])
                    y = ep.tile([P, 128], F32, tag="y")
                    nc.vector.tensor_tensor(
                        out=y[:].rearrange("p (h c) -> p h c", c=32),
                        in0=acc[:, 0:128].rearrange("p (h c) -> p h c", c=32),
                        in1=rcp[:, :, None].to_broadcast([P, 4, 32]),
                        op=ALU.mult,
                    )
                    y2 = ep.tile([P, 128], F32, tag="y2")
                    nc.vector.tensor_tensor(
                        out=y2[:], in0=y[:],
                        in1=skip_sb[:, b * P:(b + 1) * P], op=ALU.add)
                    # elu(v) = max(v,0) + exp(min(v,0)) - 1
                    mn = ep.tile([P, 128], F32, tag="mn")
                    nc.vector.tensor_scalar_min(out=mn[:], in0=y2[:],
                                                scalar1=0.0)
                    e1 = ep.tile([P, 128], F32, tag="e1")
                    nc.scalar.activation(out=e1[:], in_=mn[:], func=ACTF.Exp)
                    mx = ep.tile([P, 128], F32, tag="mx")
                    nc.vector.tensor_scalar_max(out=mx[:], in0=y2[:],
                                                scalar1=0.0)
                    yo = ep.tile([P, 128], F32, tag="yo")
                    nc.vector.scalar_tensor_tensor(
                        out=yo[:], in0=mx[:], scalar=-1.0, in1=e1[:],
                        op0=ALU.add, op1=ALU.add)
                    nc.scalar.dma_start(out=out[b * P:(b + 1) * P, :],
                                        in_=yo[:])
                    aoff += TA
                    boff += TB
                    doff += T

    nc.compile()
    return nc


# ---------------------------------------------------------------------------
# Public entry point.

_CACHE = {}


def _get_program(cfg):
    key = (cfg.N, cfg.E, cfg.NC, tuple(cfg.TAS), tuple(cfg.TBS))
    if key not in _CACHE:
        _CACHE[key] = build_program(cfg)
    return _CACHE[key]


def run_full(inputs, trace=False, **spmd_kwargs):
    cfg = Cfg()
    in_maps = make_inputs(cfg, **{k: np.asarray(v) for k, v in inputs.items()})
    nc = _get_program(cfg)
    res = run_bass_kernel_spmd(nc, in_maps, list(range(cfg.NC)), trace=trace,
                               **spmd_kwargs)
    outs = [res.results[c]["out"][:cfg.NLOC] for c in range(cfg.NC)]
    return np.concatenate(outs, axis=0).astype(np.float32), res


def kernel(x, edge_index, W, att_src, att_dst, bias, skip_W, skip_b):
    out, _ = run_full(dict(x=x, edge_index=edge_index, W=W, att_src=att_src,
                           att_dst=att_dst, bias=bias, skip_W=skip_W,
                           skip_b=skip_b))
    return out


# revision 8
# speedup vs baseline: 2.3046x; 1.0102x over previous
"""GAT residual block (nn_GATResBlock) on 8 Trainium2 NeuronCores.

Strategy (v3 — SWDGE gather, trimmed + offloaded)
-------------------------------------------------
- Shard destination nodes (and their incoming edges) across the 8 cores;
  each core owns a contiguous range of 6250 dst nodes (49 blocks of 128).
- Host-side graph preprocessing: per (core, block), edges sorted and split
  into A/B halves by src row (int16 gather index range is 32768 rows),
  padded to 128-edge tiles (per-block ragged tile counts, max over cores,
  so descriptor-generation time tracks the real edge count); pad slots
  gather node row 0 and are masked out of the one-hot aggregation matrix
  via dloc = -1.
- Algebraic folds: a_src = x @ (W.T @ att_src-expanded) comes out of the
  projection matmul; the segment-softmax max-trick is dropped (logits are
  bounded, softmax is shift invariant) and normalization happens at node
  level: agg = (sum ex*xp[src]) / (sum ex).
- Device per core: one replicated projection pass (bf16 inputs, scalar-
  engine PSUM evacuation) builds a DRAM node table T1[row] = [xp | a_src];
  per dst-block, dma_gather fetches the rows of the block's source nodes,
  a one-hot (edge,dst) selection matrix built with iota/is_equal feeds a
  PSUM-accumulated matmul that reduces weighted messages + softmax
  denominators in one pass; a_dst reaches the edges via per-tile
  transposes of the one-hot matrix (PSUM copies on the scalar engine) and
  tiny matmuls. Epilogue divides, adds the skip projection, applies ELU.
"""

import sys
import types

sys.path.insert(0, "/opt/trn_rl_repo")

import numpy as np
import ml_dtypes

BFNP = ml_dtypes.bfloat16


# ---------------------------------------------------------------------------
# NTFF profile hook (missing antenv.axon_hooks in this image). Needed only
# when tracing; harmless otherwise.
def _install_ntff_hook():
    if "antenv.axon_hooks" in sys.modules:
        return
    try:
        hooks = types.ModuleType("antenv.axon_hooks")
        _h = [None]
        hooks.set_axon_ntff_profile_hook = lambda h: _h.__setitem__(0, h)
        hooks.get_axon_ntff_profile_hook = lambda: _h[0]
        sys.modules["antenv.axon_hooks"] = hooks
        import antenv

        antenv.axon_hooks = hooks
        from trn_agent_boot.trn_boot import _ntff_profile_via_ctypes

        hooks.set_axon_ntff_profile_hook(
            _ntff_profile_via_ctypes("/opt/axon/libaxon_pjrt.so")
        )
    except Exception:
        pass


_install_ntff_hook()

from concourse import bacc, mybir, tile  # noqa: E402
from concourse.bass_utils import run_bass_kernel_spmd  # noqa: E402

F32 = mybir.dt.float32
BF16 = mybir.dt.bfloat16
I16 = mybir.dt.int16
ALU = mybir.AluOpType
ACTF = mybir.ActivationFunctionType

P = 128
NEG_SLOPE = 0.2
ROWW = 256  # T1 bf16 cols: xp(128) + a_src(4) + pad (elem must be 256B mult)


class Cfg:
    def __init__(self, N=50000, IN=128, H=4, C=32, E=800000, NC=8,
                 SPLIT=32768):
        self.N, self.IN, self.H, self.C, self.E, self.NC = N, IN, H, C, E, NC
        self.HC = H * C
        assert self.HC == 128 and IN == 128
        assert N % NC == 0
        self.NLOC = N // NC                      # owned dst nodes per core
        self.NBLK = (self.NLOC + P - 1) // P     # dst blocks per core
        self.NLOCP = self.NBLK * P               # padded local nodes
        self.SPLIT = SPLIT                       # int16 A/B table split
        self.NR = ((N + P - 1) // P) * P         # node-table rows (padded)
        assert self.NR - SPLIT <= 32768
        self.TAS = None   # per-block A tiles
        self.TBS = None   # per-block B tiles


# ---------------------------------------------------------------------------
# Host-side preprocessing: edge partitioning + gather index construction.


def _wrap_idx(arr):
    """[K*128] edge-slot array -> [128, K*8] int16 'wrapped' index layout
    (index i lives at [i % 16, i // 16], replicated across the 8 groups)."""
    k16 = arr.reshape(-1, 16).T.astype(np.int16)  # [16, K*8]
    return np.tile(k16, (8, 1))                   # [128, K*8]


def preprocess(cfg, edge_index):
    """Build per-core gather index arrays from the (2, E) edge list."""
    src = np.asarray(edge_index[0], dtype=np.int64)
    dst = np.asarray(edge_index[1], dtype=np.int64)
    core = dst // cfg.NLOC
    dstl = dst - core * cfg.NLOC
    blk = dstl // P
    isB = (src >= cfg.SPLIT).astype(np.int64)

    order = np.lexsort((src, isB, blk, core))
    core_s, blk_s, isB_s = core[order], blk[order], isB[order]
    src_s, dstl_s = src[order], dstl[order]

    gid = ((core_s * cfg.NBLK) + blk_s) * 2 + isB_s
    ngroups = cfg.NC * cfg.NBLK * 2
    counts = np.bincount(gid, minlength=ngroups)
    starts = np.concatenate(([0], np.cumsum(counts)[:-1]))
    rank = np.arange(len(gid)) - starts[gid]

    cAB = counts.reshape(cfg.NC, cfg.NBLK, 2)
    tas = np.maximum(1, -(-cAB[:, :, 0].max(axis=0) // P))   # [NBLK]
    tbs = np.maximum(1, -(-cAB[:, :, 1].max(axis=0) // P))
    cfg.TAS, cfg.TBS = tas.tolist(), tbs.tolist()
    aoff = np.concatenate(([0], np.cumsum(tas)[:-1]))
    boff = np.concatenate(([0], np.cumsum(tbs)[:-1]))
    doff = np.concatenate(([0], np.cumsum(tas + tbs)[:-1]))
    SA, SB = int(tas.sum()), int(tbs.sum())
    ST = SA + SB

    idxA = np.zeros((cfg.NC, SA * P), dtype=np.int64)
    idxB = np.zeros((cfg.NC, SB * P), dtype=np.int64)
    dloc = np.full((cfg.NC, P, ST), -1.0, dtype=np.float32)

    a = isB_s == 0
    idxA[core_s[a], aoff[blk_s[a]] * P + rank[a]] = src_s[a]
    ta = rank[a] // P
    dloc[core_s[a], rank[a] % P, doff[blk_s[a]] + ta] = \
        dstl_s[a] - blk_s[a] * P
    b = ~a
    idxB[core_s[b], boff[blk_s[b]] * P + rank[b]] = src_s[b] - cfg.SPLIT
    tb = rank[b] // P
    dloc[core_s[b], rank[b] % P,
         doff[blk_s[b]] + tas[blk_s[b]] + tb] = dstl_s[b] - blk_s[b] * P

    per_core = []
    for c in range(cfg.NC):
        wA = np.concatenate(
            [_wrap_idx(idxA[c, aoff[b2] * P:(aoff[b2] + tas[b2]) * P])
             for b2 in range(cfg.NBLK)], axis=1)
        wB = np.concatenate(
            [_wrap_idx(idxB[c, boff[b2] * P:(boff[b2] + tbs[b2]) * P])
             for b2 in range(cfg.NBLK)], axis=1)
        per_core.append(dict(idxA=np.ascontiguousarray(wA),
                             idxB=np.ascontiguousarray(wB),
                             dloc=np.ascontiguousarray(dloc[c])))
    return per_core


def make_weights(cfg, W, att_src, att_dst, bias, skip_W, skip_b):
    """Fold attention vectors into the projection weights."""
    H, C, IN = cfg.H, cfg.C, cfg.IN
    A_s = np.zeros((IN, H), dtype=np.float32)
    A_d = np.zeros((IN, H), dtype=np.float32)
    for h in range(H):
        # a_src[n,h] = sum_c xp[n,h*C+c]*att_src[h,c] = x @ (W[h*C:+C].T @ att)
        A_s[:, h] = W[h * C:(h + 1) * C, :].T @ att_src[0, h]
        A_d[:, h] = W[h * C:(h + 1) * C, :].T @ att_dst[0, h]
    Wcat = np.concatenate([W.T, A_s], axis=1).astype(BFNP)       # [IN,132]
    Wsk = np.concatenate([skip_W.T, A_d], axis=1).astype(BFNP)   # [IN,132]
    bias2 = np.tile((bias + skip_b).astype(np.float32)[None, :], (P, 1))
    return Wcat, Wsk, bias2


def make_inputs(cfg, x, edge_index, W, att_src, att_dst, bias, skip_W, skip_b):
    per_core_idx = preprocess(cfg, edge_index)
    Wcat, Wsk, bias2 = make_weights(cfg, W, att_src, att_dst, bias, skip_W,
                                    skip_b)
    xT = np.zeros((cfg.IN, cfg.NR), dtype=BFNP)
    xT[:, 0:cfg.N] = np.asarray(x, dtype=np.float32).T.astype(BFNP)
    iota = np.tile(np.arange(P, dtype=np.float32)[None, :], (P, 1))
    iotap = np.tile(np.arange(P, dtype=np.float32)[:, None], (1, P))

    in_maps = []
    for c in range(cfg.NC):
        xTl = np.zeros((cfg.IN, cfg.NLOCP), dtype=BFNP)
        xTl[:, :cfg.NLOC] = np.asarray(
            x[c * cfg.NLOC:(c + 1) * cfg.NLOC],
            dtype=np.float32).T.astype(BFNP)
        m = dict(xT=xT, xTl=np.ascontiguousarray(xTl), Wcat=Wcat, Wsk=Wsk,
                 bias2=bias2, iota=iota, iotap=iotap, **per_core_idx[c])
        in_maps.append(m)
    return in_maps


# ---------------------------------------------------------------------------
# Device program.


def build_program(cfg):
    """Build the per-core SPMD Bass program."""
    nc = bacc.Bacc(None)
    NBLK, NR = cfg.NBLK, cfg.NR
    TAS, TBS = cfg.TAS, cfg.TBS
    SA, SB = sum(TAS), sum(TBS)
    ST = SA + SB
    TMAX = max(TAS[b] + TBS[b] for b in range(NBLK))

    xT = nc.declare_dram_parameter("xT", [cfg.IN, NR], BF16, isOutput=False)
    xTl = nc.declare_dram_parameter("xTl", [cfg.IN, cfg.NLOCP], BF16,
                                    isOutput=False)
    Wcat = nc.declare_dram_parameter("Wcat", [cfg.IN, 132], BF16,
                                     isOutput=False)
    Wsk = nc.declare_dram_parameter("Wsk", [cfg.IN, 132], BF16,
                                    isOutput=False)
    bias2 = nc.declare_dram_parameter("bias2", [P, 128], F32, isOutput=False)
    iota = nc.declare_dram_parameter("iota", [P, P], F32, isOutput=False)
    iotap = nc.declare_dram_parameter("iotap", [P, P], F32, isOutput=False)
    idxA = nc.declare_dram_parameter("idxA", [P, SA * 8], I16, isOutput=False)
    idxB = nc.declare_dram_parameter("idxB", [P, SB * 8], I16, isOutput=False)
    dloc = nc.declare_dram_parameter("dloc", [P, ST], F32, isOutput=False)
    out = nc.declare_dram_parameter("out", [cfg.NLOCP, 128], F32,
                                    isOutput=True)

    T1 = nc.dram_tensor("T1", [NR, ROWW], BF16)

    with tile.TileContext(nc) as tc:
        with (
            tc.tile_pool(name="const", bufs=1) as cpool,
            tc.tile_pool(name="prol", bufs=4) as prol,
            tc.tile_pool(name="gbuf", bufs=4) as gb,
            tc.tile_pool(name="main", bufs=4) as mp,
            tc.tile_pool(name="epi", bufs=2) as ep,
        ):
            # ---- constants ----
            iota_sb = cpool.tile([P, P], F32)
            nc.sync.dma_start(out=iota_sb[:], in_=iota[:])
            iotap_sb = cpool.tile([P, P], F32)
            nc.sync.dma_start(out=iotap_sb[:], in_=iotap[:])
            ident_bf = cpool.tile([P, P], BF16)
            nc.vector.tensor_tensor(out=ident_bf[:], in0=iota_sb[:],
                                    in1=iotap_sb[:], op=ALU.is_equal)
            wcat_sb = cpool.tile([P, 132], BF16)
            nc.sync.dma_start(out=wcat_sb[:], in_=Wcat[:])
            wsk_sb = cpool.tile([P, 132], BF16)
            nc.sync.dma_start(out=wsk_sb[:], in_=Wsk[:])
            bias_sb = cpool.tile([P, 128], F32)
            nc.sync.dma_start(out=bias_sb[:], in_=bias2[:])
            idxA_sb = cpool.tile([P, SA * 8], I16)
            nc.sync.dma_start(out=idxA_sb[:], in_=idxA[:])
            idxB_sb = cpool.tile([P, SB * 8], I16)
            nc.sync.dma_start(out=idxB_sb[:], in_=idxB[:])
            dloc_sb = cpool.tile([P, ST], F32)
            nc.sync.dma_start(out=dloc_sb[:], in_=dloc[:])
            skip_sb = cpool.tile([P, NBLK * 128], F32)
            adst_sb = cpool.tile([P, NBLK * 4], BF16)

            xl_all = cpool.tile([P, cfg.NLOCP], BF16)
            nc.sync.dma_start(out=xl_all[:], in_=xTl[:])

            # ---- phase 1: global node table T1 = [xp(bf16) | a_src] ----
            with tc.tile_pool(name="pp", bufs=2, space="PSUM") as pp:
                # ---- phase 2 first: local skip projection + a_dst (SBUF) ----
                for j in range(NBLK):
                    ps2 = pp.tile([P, 132], F32, tag="ps2")
                    nc.tensor.matmul(out=ps2[:],
                                     lhsT=xl_all[:, j * P:(j + 1) * P],
                                     rhs=wsk_sb[:], start=True, stop=True)
                    nc.vector.tensor_tensor(out=skip_sb[:, j * P:(j + 1) * P],
                                            in0=ps2[:, 0:128], in1=bias_sb[:],
                                            op=ALU.add)
                    nc.scalar.activation(out=adst_sb[:, j * 4:(j + 1) * 4],
                                         in_=ps2[:, 128:132], func=ACTF.Copy)
                CH = 8
                for i0 in range(0, NR // P, CH):
                    ch = min(CH, NR // P - i0)
                    xt = prol.tile([P, CH * P], BF16, tag="xt")
                    nc.sync.dma_start(
                        out=xt[:, 0:ch * P], in_=xT[:, i0 * P:(i0 + ch) * P])
                    st4 = prol.tile([P, CH, 132], BF16, tag="st4")
                    for k in range(0, ch, 2):
                        kn = min(2, ch - k)
                        ps = pp.tile([P, 2, 132], F32, tag="ps")
                        for j in range(kn):
                            nc.tensor.matmul(
                                out=ps[:, j, :],
                                lhsT=xt[:, (k + j) * P:(k + j + 1) * P],
                                rhs=wcat_sb[:], start=True, stop=True)
                        nc.scalar.activation(out=st4[:, k:k + kn, :],
                                             in_=ps[:, 0:kn, :],
                                             func=ACTF.Copy)
                    nc.scalar.dma_start(
                        out=T1[i0 * P:(i0 + ch) * P, 0:132].rearrange(
                            "(k p) c -> p k c", p=P),
                        in_=st4[:, 0:ch, :])

                # ---- phase 2: local skip projection + a_dst (SBUF) ----
                for j in range(NBLK):
                    xlb = prol.tile([P, P], BF16, tag="xlb")
                    nc.sync.dma_start(out=xlb[:],
                                      in_=xTl[:, j * P:(j + 1) * P])
                    ps2 = pp.tile([P, 132], F32, tag="ps2")
                    nc.tensor.matmul(out=ps2[:], lhsT=xlb[:], rhs=wsk_sb[:],
                                     start=True, stop=True)
                    nc.vector.tensor_tensor(out=skip_sb[:, j * P:(j + 1) * P],
                                            in0=ps2[:, 0:128], in1=bias_sb[:],
                                            op=ALU.add)
                    nc.vector.tensor_copy(out=adst_sb[:, j * 4:(j + 1) * 4],
                                          in_=ps2[:, 128:132])

            # ---- phase 3: per-dst-block edge processing ----
            aoff = boff = doff = 0
            with (
                tc.tile_pool(name="acc", bufs=2, space="PSUM") as ap,
                tc.tile_pool(name="stp", bufs=2, space="PSUM") as sp,
                tc.tile_pool(name="adp", bufs=2, space="PSUM") as adp,
            ):
                for b in range(NBLK):
                    TA, TB = TAS[b], TBS[b]
                    T = TA + TB
                    G1 = gb.tile([P, TMAX, ROWW], BF16, tag="G1")
                    nc.gpsimd.dma_gather(
                        out_ap=G1[:, 0:TA, :],
                        in_ap=T1[0:cfg.SPLIT, :],
                        idxs_ap=idxA_sb[:, aoff * 8:(aoff + TA) * 8],
                        num_idxs=TA * P,
                        num_idxs_reg=TA * P,
                        elem_size=ROWW,
                        single_packet=False,
                    )
                    nc.gpsimd.dma_gather(
                        out_ap=G1[:, TA:T, :],
                        in_ap=T1[cfg.SPLIT:, :],
                        idxs_ap=idxB_sb[:, boff * 8:(boff + TB) * 8],
                        num_idxs=TB * P,
                        num_idxs_reg=TB * P,
                        elem_size=ROWW,
                        single_packet=False,
                    )

                    # one-hot S[e, d] = (dloc[e] == d), bf16; pads (-1) -> 0
                    S = mp.tile([P, TMAX, P], BF16, tag="S")
                    nc.vector.tensor_tensor(
                        out=S[:, 0:T, :],
                        in0=dloc_sb[:, doff:doff + T,
                                    None].to_broadcast([P, T, P]),
                        in1=iota_sb[:, None, :].to_broadcast([P, T, P]),
                        op=ALU.is_equal,
                    )
                    # a_dst -> per-edge via St = S^T + tiny matmul
                    elog = mp.tile([P, TMAX, 4], F32, tag="elog")
                    for t0 in range(0, T, 4):
                        tn = min(4, T - t0)
                        stps = sp.tile([P, 512], BF16, tag="stps")
                        for k in range(tn):
                            nc.tensor.transpose(
                                out=stps[:, k * P:(k + 1) * P],
                                in_=S[:, t0 + k, :], identity=ident_bf[:])
                        st4b = mp.tile([P, 4, P], BF16, tag="st4b")
                        nc.scalar.activation(out=st4b[:, 0:tn, :],
                                             in_=stps[:, 0:tn * P].rearrange(
                                                 "p (t c) -> p t c", c=P),
                                             func=ACTF.Copy)
                        adps = adp.tile([P, 16], F32, tag="adps")
                        for k in range(tn):
                            nc.tensor.matmul(
                                out=adps[:, k * 4:(k + 1) * 4],
                                lhsT=st4b[:, k, :],
                                rhs=adst_sb[:, b * 4:(b + 1) * 4],
                                start=True, stop=True)
                        for (u0, un, p0) in (
                                [(t0, tn, 0)] if (t0 >= TA or t0 + tn <= TA)
                                else [(t0, TA - t0, 0),
                                      (TA, t0 + tn - TA, TA - t0)]):
                            nc.vector.tensor_tensor(
                                out=elog[:, u0:u0 + un, :],
                                in0=G1[:, u0:u0 + un, 128:132],
                                in1=adps[:, p0 * 4:(p0 + un) * 4].rearrange(
                                    "p (t f) -> p t f", f=4),
                                op=ALU.add)
                    el2 = mp.tile([P, TMAX, 4], F32, tag="el2")
                    nc.vector.scalar_tensor_tensor(
                        out=el2[:, 0:T, :], in0=elog[:, 0:T, :],
                        scalar=NEG_SLOPE, in1=elog[:, 0:T, :],
                        op0=ALU.mult, op1=ALU.max)
                    ex = mp.tile([P, TMAX, 4], F32, tag="ex")
                    nc.scalar.activation(out=ex[:, 0:T, :], in_=el2[:, 0:T, :],
                                         func=ACTF.Exp)
                    # V = [ex * xp | ex]  (bf16)
                    V = mp.tile([P, TMAX, 132], BF16, tag="V")
                    nc.vector.tensor_tensor(
                        out=V[:, 0:T, 0:128].rearrange(
                            "p t (h c) -> p t h c", c=32),
                        in0=G1[:, 0:T, 0:128].rearrange(
                            "p t (h c) -> p t h c", c=32),
                        in1=ex[:, 0:T, :, None].to_broadcast([P, T, 4, 32]),
                        op=ALU.mult,
                    )
                    nc.scalar.activation(out=V[:, 0:T, 128:132],
                                         in_=ex[:, 0:T, :], func=ACTF.Copy)
                    acc = ap.tile([P, 132], F32)
                    for t in range(T):
                        nc.tensor.matmul(out=acc[:], lhsT=S[:, t, :],
                                         rhs=V[:, t, :], start=(t == 0),
                                         stop=(t == T - 1))
                    # epilogue: divide, + skip, ELU
                    dn = ep.tile([P, 4], F32, tag="dn")
                    nc.vector.tensor_scalar_add(out=dn[:],
                                                in0=acc[:, 128:132],
                                                scalar1=1e-6)
                    rcp = ep.tile([P, 4], F32, tag="rcp")
                    nc.vector.reciprocal(out=rcp[:], in_=dn[:])
                    y = ep.tile([P, 128], F32, tag="y")
                    nc.vector.tensor_tensor(
                        out=y[:].rearrange("p (h c) -> p h c", c=32),
                        in0=acc[:, 0:128].rearrange("p (h c) -> p h c", c=32),
                        in1=rcp[:, :, None].to_broadcast([P, 4, 32]),
                        op=ALU.mult,
                    )
                    y2 = ep.tile([P, 128], F32, tag="y2")
                    nc.vector.tensor_tensor(
                        out=y2[:], in0=y[:],
                        in1=skip_sb[:, b * P:(b + 1) * P], op=ALU.add)
                    # elu(v) = max(v,0) + exp(min(v,0)) - 1
                    mn = ep.tile([P, 128], F32, tag="mn")
                    nc.vector.tensor_scalar_min(out=mn[:], in0=y2[:],
                                                scalar1=0.0)
                    e1 = ep.tile([P, 128], F32, tag="e1")
                    nc.scalar.activation(out=e1[:], in_=mn[:], func=ACTF.Exp)
                    mx = ep.tile([P, 128], F32, tag="mx")
                    nc.vector.tensor_scalar_max(out=mx[:], in0=y2[:],
                                                scalar1=0.0)
                    yo = ep.tile([P, 128], F32, tag="yo")
                    nc.vector.scalar_tensor_tensor(
                        out=yo[:], in0=mx[:], scalar=-1.0, in1=e1[:],
                        op0=ALU.add, op1=ALU.add)
                    nc.scalar.dma_start(out=out[b * P:(b + 1) * P, :],
                                        in_=yo[:])
                    aoff += TA
                    boff += TB
                    doff += T

    nc.compile()
    return nc


# ---------------------------------------------------------------------------
# Public entry point.

_CACHE = {}


def _get_program(cfg):
    key = (cfg.N, cfg.E, cfg.NC, tuple(cfg.TAS), tuple(cfg.TBS))
    if key not in _CACHE:
        _CACHE[key] = build_program(cfg)
    return _CACHE[key]


def run_full(inputs, trace=False, **spmd_kwargs):
    cfg = Cfg()
    in_maps = make_inputs(cfg, **{k: np.asarray(v) for k, v in inputs.items()})
    nc = _get_program(cfg)
    res = run_bass_kernel_spmd(nc, in_maps, list(range(cfg.NC)), trace=trace,
                               **spmd_kwargs)
    outs = [res.results[c]["out"][:cfg.NLOC] for c in range(cfg.NC)]
    return np.concatenate(outs, axis=0).astype(np.float32), res


def kernel(x, edge_index, W, att_src, att_dst, bias, skip_W, skip_b):
    out, _ = run_full(dict(x=x, edge_index=edge_index, W=W, att_src=att_src,
                           att_dst=att_dst, bias=bias, skip_W=skip_W,
                           skip_b=skip_b))
    return out
